# revision 1
# baseline (speedup 1.0000x reference)
"""kNN-accuracy (retrieval_knn) Trainium2 Bass kernel — 8-core SPMD.

Problem: acc = masked mean over n of [target[n] in top-K nearest word_vectors
to logits[n]] with N=4096, D=512, V=50000, K=10.

Algorithm (screen + exact refine, no top-k, no collectives):
  target is in the top-K  <=>  #{v: d2[n,v] < d2[n,target[n]]} < K.
  x^2 cancels in the comparison, so rank by s[n,v] = w2[v] - 2*x_n.w_v.

  A count over ANY subset of the vocab is a sound lower bound on the full
  count: if a row's subset count already reaches K the row is a certain
  miss.  Per core (vocab shard of VS=6272 columns):

  - SCREEN: bf16 matmul of all N rows against the first SUB_W=512 words of
    the core's shard, plus an exact fp32r w2 add; count words with
    s < T_lo[n] where T_lo = T_host - delta is a host-computed sound lower
    bound on the on-device threshold (delta covers the bf16 quantization +
    accumulation error, validated on data: max err 0.69 < 1.0).
    Rows with screen count <= K are "risky" (~120-150 of 4096 per core).
  - COMPACT on device: risky flags -> prefix sum via triangular matmuls ->
    slot list via is_eq(iota, slot) selection matmuls (values n+1, 0=empty).
  - GATHER: indirect DMA fetches the risky rows' x / target-word vectors
    (fp32r) from DRAM; PE transposes them into stationary layout.
  - REFINE: exact fp32r pass of <=NSLOT=256 gathered rows over the FULL
    shard.  The per-row threshold T is re-extracted on device with the
    same stationary tiles, chunk order and PSUM accumulation order as the
    refine matmuls, so s[j, target[j]] == T[j] bit-identically and the
    strict-< count gets exact boundary semantics (baseline's trick).
  - HOST combine: a row is a hit iff every core refined it (guaranteed for
    any true hit: subset count <= full count < K on every core) and the
    summed refined counts < K.  All other rows are proven misses.

Device work per core ~ N*SUB_W*D bf16 matmul + NSLOT*VS*D fp32r matmul;
HBM traffic ~ 4MB (bf16 x) + 12.8MB (fp32r shard) vs 21MB+ for the full
N x VS fp32r baseline, and ~7.5x fewer matmul cycles.
"""
import sys

for _p in ("/opt/trn_rl_repo", "/root/.axon_site/_ro/trn_rl_repo"):
    if _p not in sys.path:
        sys.path.insert(0, _p)

import numpy as np
import ml_dtypes
import concourse.mybir as mybir
import concourse.tile as tile
from concourse import bacc, bass
from contextlib import ExitStack

N = 4096
D = 512
V = 50000
K = 10
NUM_CORES = 8
VT = 448                 # refine matmul moving free dim (v-tile width)
TPC = 14                 # refine v-tiles per core
VS = VT * TPC            # 6272 vocab columns per core
VPAD = VS * NUM_CORES    # 50176
NT = N // 128            # 32 n-tiles
DC = D // 128            # 4 contraction chunks
SUB_W = 640              # screen subset width per core (512 + 128 split)
NSLOT = 128              # refine row capacity (1 tile of 128)
NST = NSLOT // 128       # refine slot tiles
DELTA = 1.0              # screen threshold safety band
RISK_THR = 10.9          # risky iff screen count-estimate < this (= K+1 fuzz-safe)
AUGW = 1280              # gather row: [x(512)|pad|wg at 640(512)|w2 at 1152|pad]

F32 = mybir.dt.float32
F32R = mybir.dt.float32r
BF16 = mybir.dt.bfloat16
I32 = mybir.dt.int32


def _round_fp32r(x):
    hi = x.astype(ml_dtypes.bfloat16).astype(np.float32)
    lo = (x - hi).astype(ml_dtypes.bfloat16).astype(np.float32)
    return hi + lo


def host_prep(logits, target, mask, word_vectors):
    """Shard/stage the full inputs into one input map per core."""
    x = np.asarray(logits, dtype=np.float32)
    t = np.asarray(target).astype(np.int64)
    W = np.asarray(word_vectors, dtype=np.float32)

    # padded vocab: zero vectors with huge norm never count
    Wp = np.zeros((VPAD, D), dtype=np.float32)
    Wp[:V] = W
    w2 = np.zeros((VPAD,), dtype=np.float32)
    w2[:V] = (W.astype(np.float64) ** 2).sum(axis=1).astype(np.float32)
    w2[V:] = 1e30

    Wr = _round_fp32r(Wp)                                # [VPAD, D]
    w2r = _round_fp32r(w2)
    xm2r = _round_fp32r(np.ascontiguousarray(-2.0 * x))  # [N, D] (-2x, fp32r)
    xb = (-2.0 * x).astype(ml_dtypes.bfloat16).astype(np.float32)  # bf16(-2x)
    Wb = Wp.astype(ml_dtypes.bfloat16).astype(np.float32)

    # host threshold, fp64: T[n] = w2r[t] - 2 x_r . w_r[t]  (sound w/ DELTA)
    T64 = (w2r[t].astype(np.float64)
           + np.einsum('nd,nd->n', xm2r.astype(np.float64),
                       Wr[t].astype(np.float64)))
    Tlo = (T64 - DELTA).astype(np.float32)

    def chunkT(a, cols):
        # [D, cols] -> [128, DC, cols]
        return np.ascontiguousarray(a.reshape(DC, 128, cols).transpose(1, 0, 2))

    xTb = chunkT(np.ascontiguousarray(xb.T), N).astype(ml_dtypes.bfloat16)
    tlo_t = np.ascontiguousarray(Tlo.reshape(NT, 128).T)  # [128, NT]

    # gather source: row n = [-2x (fp32r) | pad | W[t] (fp32r) | w2[t] | pad]
    aug = np.zeros((N, AUGW), dtype=np.float32)
    aug[:, :D] = xm2r
    aug[:, 640:640 + D] = Wr[t]
    aug[:, 1152] = w2r[t]

    # constants
    idm = np.eye(128, dtype=np.float32)
    ones1 = np.zeros((128, 128), dtype=np.float32)
    for g in range(4):
        ones1[32 * g, :] = 1.0
    iotaJ = np.tile(np.arange(NSLOT, dtype=np.float32), (128, 1))   # [128, NSLOT]
    iotaN1 = np.ascontiguousarray(np.repeat(
        (np.arange(N, dtype=np.float32) + 1.0).reshape(NT, 128).T[:, :, None],
        2, axis=2))  # [128, NT, 2] duplicated pair (fp32r needs even free)
    tri128 = np.tril(np.ones((128, 128), dtype=np.float32), -1).T   # [p', p]=1 if p'<p
    tris = np.zeros((128, 32), dtype=np.float32)
    tris[:32] = np.tril(np.ones((32, 32), dtype=np.float32), -1).T  # [i', i]=1 if i'<i
    onesc = np.ones((128, 2), dtype=np.float32)
    onerow = np.ones((1, 128), dtype=np.float32)

    common = dict(xTb=xTb, tlo=tlo_t, aug=aug, idm=idm,
                  ones1=ones1, iotaJ=iotaJ, iotaN1=iotaN1, tri128=tri128,
                  tris=tris, onesc=onesc, onerow=onerow)
    in_maps = []
    for c in range(NUM_CORES):
        sl = slice(c * VS, (c + 1) * VS)
        m = dict(common)
        m["wT"] = chunkT(np.ascontiguousarray(Wr[sl].T), VS)
        m["wsub"] = chunkT(np.ascontiguousarray(
            Wb[c * VS:c * VS + SUB_W].T), SUB_W).astype(ml_dtypes.bfloat16)
        # screen w2 (exact fp32r values) at rows {0,32,64,96} (g = i%4)
        w2s = np.zeros((128, SUB_W), dtype=np.float32)
        for g in range(4):
            w2s[32 * g, :] = w2r[c * VS:c * VS + SUB_W]
        m["w2s"] = w2s
        # refine w2 pack: row 32*(v%4), block v//4
        w2p = np.zeros((128, (TPC + 3) // 4, VT), dtype=np.float32)
        for v in range(TPC):
            w2p[32 * (v % 4), v // 4, :] = w2r[c * VS + v * VT:c * VS + (v + 1) * VT]
        m["w2p"] = w2p
        in_maps.append(m)
    return in_maps


def build_nc(num_cores=NUM_CORES):
    nc = bacc.Bacc("TRN2", target_bir_lowering=False, debug=False,
                   num_devices=num_cores)
    ins = {
        "xTb": nc.dram_tensor("xTb", [128, DC, N], BF16, kind="ExternalInput").ap(),
        "wT": nc.dram_tensor("wT", [128, DC, VS], F32, kind="ExternalInput").ap(),
        "wsub": nc.dram_tensor("wsub", [128, DC, SUB_W], BF16, kind="ExternalInput").ap(),
        "w2s": nc.dram_tensor("w2s", [128, SUB_W], F32, kind="ExternalInput").ap(),
        "w2p": nc.dram_tensor("w2p", [128, (TPC + 3) // 4, VT], F32, kind="ExternalInput").ap(),
        "tlo": nc.dram_tensor("tlo", [128, NT], F32, kind="ExternalInput").ap(),
        "aug": nc.dram_tensor("aug", [N, AUGW], F32, kind="ExternalInput").ap(),
        "idm": nc.dram_tensor("idm", [128, 128], F32, kind="ExternalInput").ap(),
        "ones1": nc.dram_tensor("ones1", [128, 128], F32, kind="ExternalInput").ap(),
        "iotaJ": nc.dram_tensor("iotaJ", [128, NSLOT], F32, kind="ExternalInput").ap(),
        "iotaN1": nc.dram_tensor("iotaN1", [128, NT, 2], F32, kind="ExternalInput").ap(),
        "tri128": nc.dram_tensor("tri128", [128, 128], F32, kind="ExternalInput").ap(),
        "tris": nc.dram_tensor("tris", [128, 32], F32, kind="ExternalInput").ap(),
        "onesc": nc.dram_tensor("onesc", [128, 2], F32, kind="ExternalInput").ap(),
        "onerow": nc.dram_tensor("onerow", [1, 128], F32, kind="ExternalInput").ap(),
    }
    outs = {
        "riskyvals": nc.dram_tensor("riskyvals", [128, NST], F32, kind="ExternalOutput").ap(),
        "cntref": nc.dram_tensor("cntref", [128, 2 * NST], F32, kind="ExternalOutput").ap(),
        "riskytot": nc.dram_tensor("riskytot", [1, 1], F32, kind="ExternalOutput").ap(),
    }
    with tile.TileContext(nc, trace_sim=False) as tc:
        _knn_kernel(tc, ins, outs)
    nc.compile()
    return nc


def _knn_kernel(tc, ins, outs, repeats=1, phases=("screen", "compact", "gather", "refine")):
    nc = tc.nc
    ctx = ExitStack()
    with ctx:
        const = ctx.enter_context(tc.tile_pool(name="const", bufs=1))
        scratch = ctx.enter_context(tc.tile_pool(name="scratch", bufs=3))
        psm = ctx.enter_context(tc.tile_pool(name="psm", bufs=5, space="PSUM"))
        psg = ctx.enter_context(tc.tile_pool(name="psg", bufs=2, space="PSUM"))
        psc = ctx.enter_context(tc.tile_pool(name="psc", bufs=1, space="PSUM"))

        # small constants: DMA'd once (negligible bytes)
        tlo_t = const.tile([128, NT], F32)
        nc.sync.dma_start(tlo_t[:], ins["tlo"][:])
        idm_t = const.tile([128, 128], F32)
        nc.sync.dma_start(idm_t[:], ins["idm"][:])
        ones_r = const.tile([128, 128], F32R)
        nc.sync.dma_start(ones_r[:], ins["ones1"].bitcast(F32R))
        iotaJ_t = const.tile([128, NSLOT], F32)
        nc.sync.dma_start(iotaJ_t[:], ins["iotaJ"][:])
        iotaN1_r = const.tile([128, NT, 2], F32R)
        nc.sync.dma_start(iotaN1_r[:], ins["iotaN1"].bitcast(F32R))
        tri_r = const.tile([128, 128], F32R)
        nc.sync.dma_start(tri_r[:], ins["tri128"].bitcast(F32R))
        tris_r = const.tile([128, 32], F32R)
        nc.sync.dma_start(tris_r[:], ins["tris"].bitcast(F32R))
        onesc_r = const.tile([128, 2], F32R)
        nc.sync.dma_start(onesc_r[:], ins["onesc"].bitcast(F32R))
        onerow_r = const.tile([1, 128], F32R)
        nc.sync.dma_start(onerow_r[:], ins["onerow"].bitcast(F32R))
        w2s_r = const.tile([128, SUB_W], F32R)
        nc.sync.dma_start(w2s_r[:], ins["w2s"].bitcast(F32R))
        w2p_r = const.tile([128, (TPC + 3) // 4, VT], F32R)
        nc.sync.dma_start(w2p_r[:], ins["w2p"].bitcast(F32R))

        for rep in range(repeats):
            # big inputs re-DMA'd per rep so the repeat-slope timing method
            # charges the HBM streaming to every iteration (honest steady
            # state); for the production build repeats == 1.
            wsub_b = const.tile([128, DC, SUB_W], BF16, tag="wsub")
            nc.sync.dma_start(wsub_b[:], ins["wsub"][:])
            xTb_t = const.tile([128, DC, N], BF16, tag="xTb")
            for q in range(8):  # split so screen x spreads over DMA queues
                sl = slice(q * (N // 8), (q + 1) * (N // 8))
                nc.sync.dma_start(xTb_t[:, :, sl], ins["xTb"][:, :, sl])
            wT_r = const.tile([128, DC, VS], F32R, tag="wT")
            for v in range(TPC):  # per-v-tile DMAs: refine consumes in order
                sl = slice(v * VT, (v + 1) * VT)
                nc.sync.dma_start(wT_r[:, :, sl], ins["wT"].bitcast(F32R)[:, :, sl])
            _knn_body(tc, ins, outs, const, scratch, psm, psg, psc, tlo_t,
                      idm_t, ones_r, iotaJ_t, iotaN1_r, tri_r, tris_r,
                      onesc_r, onerow_r, w2s_r, wsub_b, xTb_t, w2p_r, wT_r,
                      phases=phases)
        if "refine" not in phases:
            # ablation builds: touch the outputs so the NEFF has writers
            dummy = const.tile([128, NST], F32, tag="dummy")
            nc.gpsimd.memset(dummy[:], 0.0)
            nc.sync.dma_start(outs["cntref"][:], dummy[:])
            if "compact" not in phases:
                nc.sync.dma_start(outs["riskyvals"][:], dummy[:])
                dummy1 = const.tile([1, 1], F32, tag="dummy1")
                nc.gpsimd.memset(dummy1[:], 0.0)
                nc.sync.dma_start(outs["riskytot"][:], dummy1[:])
            # consume the big tiles so their DMAs aren't dead-code
            sink = psc.tile([128, NT], F32, tag="pc", name="sink")
            nc.tensor.matmul(sink[0:128, 0:32], wT_r[:, 0, 0:128],
                             wT_r[:, 0, VS - 32:VS], start=True, stop=True)
            nc.tensor.matmul(sink[0:128, 0:32], xTb_t[:, 0, 0:128],
                             xTb_t[:, 0, 0:32], start=False, stop=False)
            nc.tensor.matmul(sink[0:128, 0:32], wsub_b[:, 0, 0:128],
                             wsub_b[:, 0, 0:32], start=False, stop=True)
            sinks = const.tile([128, 32], F32, tag="sinks")
            nc.vector.tensor_copy(sinks[:], sink[0:128, 0:32])


def _knn_body(tc, ins, outs, persist, scratch, psm, psg, psc, tlo_t, idm_t,
              ones_r, iotaJ_t, iotaN1_r, tri_r, tris_r, onesc_r, onerow_r,
              w2s_r, wsub_b, xTb_t, w2p_r, wT_r,
              phases=("screen", "compact", "gather", "refine")):
    nc = tc.nc
    WA, WB = 512, SUB_W - 512
    if "screen" not in phases:
        return

    # ---- phase 1: screen -------------------------------------------------
    # per i-tile: two PSUM tiles (FD 512 + FD 128); one count op on DVE
    # (exact is_lt) and one on ACT (sign trick), swapping tiles by parity
    # to balance engine load.
    cntd = persist.tile([128, NT], F32, tag="cntd")
    cnta = persist.tile([128, NT], F32, tag="cnta")
    for i0 in range(0, NT, 2):
        pms = []
        for g in range(2):
            i = i0 + g
            pa = psm.tile([128, WA], F32, tag="pm", name="pa")
            pbt = psg.tile([128, 256], F32, tag="px", name="pbt")
            pb = pbt[:, 0:WB]
            pms.append((pa, pb))
            for d in range(DC):
                nc.tensor.matmul(pa[:], xTb_t[:, d, i * 128:(i + 1) * 128],
                                 wsub_b[:, d, 0:WA], start=(d == 0), stop=False)
            for d in range(DC):
                nc.tensor.matmul(pb, xTb_t[:, d, i * 128:(i + 1) * 128],
                                 wsub_b[:, d, WA:SUB_W], start=(d == 0), stop=False)
        for g in range(2):
            i = i0 + g
            gg = i % 4
            nc.tensor.matmul(
                pms[g][0][:], ones_r[32 * gg:32 * gg + 32, :],
                w2s_r[32 * gg:32 * gg + 32, 0:WA],
                start=False, stop=True, tile_position=(32 * gg, 0))
            gg2 = (i + 2) % 4
            nc.tensor.matmul(
                pms[g][1], ones_r[32 * gg2:32 * gg2 + 32, :],
                w2s_r[32 * gg2:32 * gg2 + 32, WA:SUB_W],
                start=False, stop=True, tile_position=(32 * gg2, 0))
        for g in range(2):
            i = i0 + g
            if i % 2 == 0:
                dve_ap, wd = pms[g][0][:], WA
                act_ap, wa = pms[g][1], WB
            else:
                dve_ap, wd = pms[g][1], WB
                act_ap, wa = pms[g][0][:], WA
            cmp = scratch.tile([128, WA], F32, tag="cmp", name="cmp")
            nc.vector.tensor_scalar(
                cmp[:, 0:wd], dve_ap, tlo_t[:, i:i + 1], None,
                op0=mybir.AluOpType.is_lt, op1=mybir.AluOpType.add,
                accum_out=cntd[:, i:i + 1])
            sg = scratch.tile([128, WA], BF16, tag="sg", name="sg")
            nc.scalar.activation(
                sg[:, 0:wa], act_ap, mybir.ActivationFunctionType.Sign,
                bias=tlo_t[:, i:i + 1], scale=-1.0,
                accum_out=cnta[:, i:i + 1])

    # ACT slots hold sum(sign(Tlo-s)) over its tile; est c_lt = (acc + W)/2
    # (over-counts by c_eq/2, fuzz absorbed in RISK_THR).  ACT tile width
    # alternates by parity: even i -> WB, odd i -> WA.
    cnts = persist.tile([128, NT], F32, tag="cnts")
    nc.vector.tensor_scalar(
        cnta[:, 0:NT:2], cnta[:, 0:NT:2], 0.5, float(WB) * 0.5,
        op0=mybir.AluOpType.mult, op1=mybir.AluOpType.add)
    nc.vector.tensor_scalar(
        cnta[:, 1:NT:2], cnta[:, 1:NT:2], 0.5, float(WA) * 0.5,
        op0=mybir.AluOpType.mult, op1=mybir.AluOpType.add)
    nc.vector.tensor_tensor(cnts[:], cntd[:], cnta[:], op=mybir.AluOpType.add)

    if "compact" not in phases:
        return
    # ---- phase 2: risky flags, prefix sum, slot list ---------------------
    Ff = persist.tile([128, NT], F32, tag="Ff")
    nc.vector.tensor_scalar(Ff[:], cnts[:], RISK_THR, None,
                            op0=mybir.AluOpType.is_lt)
    Fr = persist.tile([128, NT], F32R, tag="Fr")
    nc.vector.tensor_copy(Fr[:], Ff[:])

    # column sums (transposed): cs[i] = sum_p F[p, i]
    pc1 = psc.tile([128, NT], F32, tag="pc", name="pc1")
    nc.tensor.matmul(pc1[0:32, 0:2], Fr[:], onesc_r[:], start=True, stop=True)
    cs_r = persist.tile([32, 2], F32R, tag="cs_r")
    nc.vector.tensor_copy(cs_r[:], pc1[0:32, 0:2])
    # exclusive prefix over columns: cp[i] = sum_{i'<i} cs[i']
    pc3 = psc.tile([128, NT], F32, tag="pc", name="pc3")
    nc.tensor.matmul(pc3[0:2, :], cs_r[:], tris_r[0:32, :], start=True, stop=True)
    cp_r = persist.tile([1, 32], F32R, tag="cp_r")
    nc.vector.tensor_copy(cp_r[:], pc3[0:1, :])
    # global exclusive prefix P[p, i] = cp[i] + sum_{p'<p} F[p', i]
    P_ps = psc.tile([128, NT], F32, tag="pc", name="P_ps")
    nc.tensor.matmul(P_ps[:], tri_r[:], Fr[:], start=True, stop=False)
    nc.tensor.matmul(P_ps[:], onerow_r[:], cp_r[:], start=False, stop=True)
    # off = P + (1-F)*2*NSLOT: slot for risky rows, out-of-range otherwise
    t2 = persist.tile([128, NT], F32, tag="t2")
    nc.vector.tensor_scalar(t2[:], Ff[:], -float(2 * NSLOT), float(2 * NSLOT),
                            op0=mybir.AluOpType.mult, op1=mybir.AluOpType.add)
    off = persist.tile([128, NT], F32, tag="off")
    nc.vector.tensor_tensor(off[:], P_ps[:], t2[:], op=mybir.AluOpType.add)
    # total risky count (overflow detection on host)
    pc2 = psc.tile([128, NT], F32, tag="pc", name="pc2")
    nc.tensor.matmul(pc2[0:2, 0:2], cs_r[:], onesc_r[0:32, :], start=True,
                     stop=True)
    tot_sb = persist.tile([1, 1], F32, tag="tot_sb")
    nc.vector.tensor_copy(tot_sb[:], pc2[0:1, 0:1])
    nc.sync.dma_start(outs["riskytot"][:], tot_sb[:])

    # slot list (transposed): idxrow[r, j] = sum_n (n+1) [off[n] == j]
    # one fused is_eq over all i-tiles, then cheap 2-col-stationary matmuls
    sel_m = persist.tile([128, NT, NSLOT], F32R, tag="selm")
    nc.vector.tensor_tensor(
        sel_m[:], iotaJ_t[:, None, :].broadcast_to([128, NT, NSLOT]),
        off[:].to_broadcast([128, NT, NSLOT]), op=mybir.AluOpType.is_equal)
    idx_ps = psg.tile([128, 256], F32, tag="px", name="idx_ps")
    for i in range(NT):
        nc.tensor.matmul(idx_ps[0:2, 0:NSLOT], iotaN1_r[:, i, :],
                         sel_m[:, i, :],
                         start=(i == 0), stop=(i == NT - 1))
    idxrow = persist.tile([2, NSLOT], F32, tag="idxrow")
    nc.vector.tensor_copy(idxrow[:], idx_ps[0:2, 0:NSLOT])
    idxv = persist.tile([128, NST], F32, tag="idxv")
    for st in range(NST):
        tpx = psg.tile([128, 256], F32, tag="px", name="tpx")
        nc.tensor.transpose(tpx[0:128, 0:2], idxrow[:, st * 128:(st + 1) * 128],
                            idm_t[0:2, 0:2])
        nc.vector.tensor_copy(idxv[:, st:st + 1], tpx[0:128, 0:1])
    nc.sync.dma_start(outs["riskyvals"][:], idxv[:])
    idxg = persist.tile([128, NST], I32, tag="idxg")
    nc.vector.tensor_scalar(idxg[:], idxv[:], 1.0, 1.0,
                            op0=mybir.AluOpType.max,
                            op1=mybir.AluOpType.subtract)

    if "gather" not in phases:
        return
    # ---- phase 3: gather risky rows + transpose to stationary layout ----
    # aug row: x chunks 0-3, wg chunks 5-8, w2 row at chunk 9 col 0
    xgT = persist.tile([128, NST, DC, 128], F32R, tag="xgT")
    wgT = persist.tile([128, NST, DC + 1, 128], F32R, tag="wgT")
    for st in range(NST):
        g_sb = scratch.tile([128, AUGW], F32, tag="gsb", name="gsb")
        nc.gpsimd.indirect_dma_start(
            out=g_sb[:], out_offset=None, in_=ins["aug"][:],
            in_offset=bass.IndirectOffsetOnAxis(
                ap=idxg[:, st:st + 1], axis=0))
        evac = []
        for ch in range(DC):
            evac.append((ch, xgT[:, st, ch, :]))
        for ch in range(DC + 1):
            evac.append((5 + ch, wgT[:, st, ch, :]))
        for k, (ch, dst_ap) in enumerate(evac):
            tp = psg.tile([128, 256], F32, tag="px", name="tp")
            nc.tensor.transpose(tp[:, 0:128], g_sb[:, ch * 128:(ch + 1) * 128],
                                idm_t[:])
            if k % 2 == 0:
                nc.vector.tensor_copy(dst_ap, tp[:, 0:128])
            else:
                nc.scalar.activation(dst_ap, tp[:, 0:128],
                                     mybir.ActivationFunctionType.Copy)

    # ---- phase 4: exact threshold extraction for gathered rows ----------
    Tg = persist.tile([128, NST], F32, tag="Tg")
    for st in range(NST):
        pg = psg.tile([128, 256], F32, tag="px", name="pg")
        for d in range(DC):
            nc.tensor.matmul(pg[:, 0:128], xgT[:, st, d, :], wgT[:, st, d, :],
                             start=(d == 0), stop=False)
        nc.tensor.matmul(pg[:, 0:128], ones_r[0:32, :], wgT[0:32, st, DC, :],
                         start=False, stop=True, tile_position=(0, 0))
        scr = scratch.tile([128, 128], F32, tag="scr", name="scr")
        nc.vector.tensor_tensor(scr[:], pg[:, 0:128], idm_t[:],
                                op=mybir.AluOpType.mult)
        nc.vector.tensor_reduce(Tg[:, st:st + 1], scr[:],
                                axis=mybir.AxisListType.X,
                                op=mybir.AluOpType.add)

    if "refine" not in phases:
        return
    # ---- phase 5: refine — exact fp32r counts over the full shard -------
    # counts: even v-tiles exact on DVE (is_lt), odd v-tiles on ACT via the
    # sign trick; the host combines (act_sum + 7*VT)/2 with a -0.5 self-pair
    # correction for the core that owns the row's target word.
    cref = persist.tile([128, NST, TPC], F32, tag="cref")
    for v0 in range(0, TPC, 2):
        for st in range(NST):
            vs = list(range(v0, min(v0 + 2, TPC)))
            pmr = {}
            for v in vs:
                pm = psm.tile([128, WA], F32, tag="pm", name="pmr")
                pmr[v] = pm
                for d in range(DC):
                    nc.tensor.matmul(pm[:, :VT], xgT[:, st, d, :],
                                     wT_r[:, d, v * VT:(v + 1) * VT],
                                     start=(d == 0), stop=False)
            for v in vs:
                g = v % 4
                nc.tensor.matmul(
                    pmr[v][:, :VT], ones_r[32 * g:32 * g + 32, :],
                    w2p_r[32 * g:32 * g + 32, v // 4, :],
                    start=False, stop=True, tile_position=(32 * g, 0))
            for v in vs:
                if v % 2 == 0:
                    cmp = scratch.tile([128, VT], F32, tag="cmpr", name="cmpr")
                    nc.vector.tensor_scalar(
                        cmp[:], pmr[v][:, :VT], Tg[:, st:st + 1], None,
                        op0=mybir.AluOpType.is_lt, op1=mybir.AluOpType.add,
                        accum_out=cref[:, st, v:v + 1])
                else:
                    sgr = scratch.tile([128, VT], BF16, tag="sgr", name="sgr")
                    nc.scalar.activation(
                        sgr[:], pmr[v][:, :VT],
                        mybir.ActivationFunctionType.Sign,
                        bias=Tg[:, st:st + 1], scale=-1.0,
                        accum_out=cref[:, st, v:v + 1])
    crefs = persist.tile([128, 2 * NST], F32, tag="crefs")
    for st in range(NST):
        nc.vector.tensor_reduce(crefs[:, 2 * st:2 * st + 1],
                                cref[:, st, 0:TPC:2],
                                axis=mybir.AxisListType.X,
                                op=mybir.AluOpType.add)
        nc.vector.tensor_reduce(crefs[:, 2 * st + 1:2 * st + 2],
                                cref[:, st, 1:TPC:2],
                                axis=mybir.AxisListType.X,
                                op=mybir.AluOpType.add)
    nc.sync.dma_start(outs["cntref"][:], crefs[:])


_NC_CACHE = {}


def _get_nc():
    if "nc" not in _NC_CACHE:
        _NC_CACHE["nc"] = build_nc()
    return _NC_CACHE["nc"]


def kernel(logits, target, mask, word_vectors):
    """Full inputs in, full output out (shape [1] float32)."""
    from concourse.bass_utils import run_bass_kernel_spmd

    in_maps = host_prep(logits, target, mask, word_vectors)
    nc = _get_nc()

    last_err = None
    res = None
    for attempt in range(3):
        try:
            res = run_bass_kernel_spmd(nc, in_maps, list(range(NUM_CORES)))
            break
        except Exception as e:  # transient NRT/axon failures: retry
            last_err = e
    if res is None:
        raise last_err

    # host combine: row is a hit iff refined on every core and sum(cnt) < K
    mask = np.asarray(mask).astype(np.float64)
    tgt = np.asarray(target).astype(np.int64)
    n_odd = (TPC + 1) // 2  # number of ACT-counted (odd) v-tiles
    totals = {}
    present = {}
    for c in range(NUM_CORES):
        r = res.results[c]
        assert float(np.asarray(r["riskytot"]).reshape(-1)[0]) <= NSLOT, \
            "risky row overflow — NSLOT too small"
        vals = np.asarray(r["riskyvals"]).reshape(128, NST)
        cnt = np.asarray(r["cntref"]).reshape(128, 2 * NST)
        for j in range(NSLOT):
            p, st = j % 128, j // 128
            v = int(round(float(vals[p, st])))
            if v <= 0:
                continue
            n = v - 1
            raw = float(cnt[p, 2 * st]) + (float(cnt[p, 2 * st + 1])
                                           + n_odd * VT) / 2.0
            tn = int(tgt[n])
            if c * VS <= tn < (c + 1) * VS and ((tn - c * VS) // VT) % 2 == 1:
                raw -= 0.5  # self-pair (s == T exactly) in an ACT tile
            assert abs(raw - round(raw)) < 1e-3, \
                f"non-integer refined count {raw} (unexpected exact tie)"
            totals[n] = totals.get(n, 0.0) + round(raw)
            present[n] = present.get(n, 0) + 1
    hits = np.zeros(N, dtype=np.float64)
    for n, p in present.items():
        if p == NUM_CORES and totals[n] < K:
            hits[n] = 1.0
    acc = (mask * hits).sum() / mask.sum()
    return np.asarray([acc], dtype=np.float32)



# revision 31
# speedup vs baseline: 1.0449x; 1.0449x over previous
"""kNN-accuracy (retrieval_knn) Trainium2 Bass kernel — 8-core SPMD.

Problem: acc = masked mean over n of [target[n] in top-K nearest word_vectors
to logits[n]] with N=4096, D=512, V=50000, K=10.

Algorithm (screen + exact refine, no top-k, no collectives):
  target is in the top-K  <=>  #{v: d2[n,v] < d2[n,target[n]]} < K.
  x^2 cancels in the comparison, so rank by s[n,v] = w2[v] - 2*x_n.w_v.

  Rows with mask==0 cannot affect the output, so the host compacts the
  masked rows away up front (N_eff = popcount(mask) ~ 2080 of 4096); the
  device only ever sees masked rows.

  A count over ANY subset of the vocab is a sound lower bound on the full
  count: if a row's subset count already reaches K the row is a certain
  miss.  Per core (vocab shard of VS=6272 columns):

  - SCREEN: bf16 matmul of the N_eff rows against the first SUB_W=512 words
    of the core's shard, plus an exact fp32r w2 add; count words with
    s < T_lo[n] where T_lo = T_host - DELTA is a host-computed sound lower
    bound on the on-device threshold (DELTA covers the bf16 quantization +
    accumulation error; validated max err 0.56 < 1.0 on the data).
    Rows with screen count < RISK_THR are "risky" (~50-70 of ~2080/core).
  - COMPACT on device: risky flags -> prefix sum via triangular matmuls ->
    slot list via is_eq(iota, slot) + per-i-tile selection matmuls with the
    selection matrix STATIONARY (slot index lands on the PSUM partition
    axis, so no transposes).  The compaction runs in two chunks: the first
    nt-4 i-tiles are compacted while the screen is still running on the
    last 4, so only a short chain remains after the screen.
  - GATHER: indirect DMA fetches the risky rows' x / target-word vectors
    (fp32r) from DRAM; PE transposes them into stationary layout.
  - REFINE: exact fp32r pass of <=NSLOT=128 gathered rows over the FULL
    shard.  The per-row threshold T is re-extracted on device with the
    same stationary tiles, chunk order and PSUM accumulation order as the
    refine matmuls, so s[j, target[j]] == T[j] bit-identically and the
    strict-< count gets exact boundary semantics.
  - HOST combine: a row is a hit iff every core refined it (guaranteed for
    any true hit: subset count <= full count < K on every core) and the
    summed refined counts < K.  All other rows are proven misses.

DMA emit order is tuned for the serialized-DMA pipeline: screen inputs
first (screen starts ~5us in), then the bulk of the fp32r shard stream in
chunks, with the last refine v-tiles streamed after the indirect gather so
the gather never queues behind the whole 12.8MB stream.
"""
import sys

for _p in ("/opt/trn_rl_repo", "/root/.axon_site/_ro/trn_rl_repo"):
    if _p not in sys.path:
        sys.path.insert(0, _p)

import numpy as np
import ml_dtypes
import concourse.mybir as mybir
import concourse.tile as tile
from concourse import bacc, bass
from contextlib import ExitStack

D = 512
V = 50000
K = 10
NUM_CORES = 8
VT = 448                 # refine matmul moving free dim (v-tile width)
TPC = 14                 # refine v-tiles per core
VS = VT * TPC            # 6272 vocab columns per core
VPAD = VS * NUM_CORES    # 50176
DC = D // 128            # 4 contraction chunks
SUB_W = 512              # screen subset width per core (one PSUM bank)
NSLOT = 128              # refine row capacity (1 tile of 128)
DELTA = 1.0              # screen threshold safety band
RISK_THR = 10.9          # risky iff screen count < this
AUGW = 1032              # gather row: [x(512)|wg(512)|w2 at 1024|pad]
N_ODD = (TPC + 1) // 2   # number of ACT-counted (odd) v-tiles in refine
FILL_MM = 10             # PE filler matmuls bridging the gather wait
WARM_MM = 20             # PE warmup matmuls during the input-DMA head
WT_TAIL = 8              # wT v-tiles [WT_TAIL, TPC) stream after the gather
ACT_THR = 2.0 * RISK_THR - SUB_W  # risky threshold on raw ACT sign-sums

F32 = mybir.dt.float32
F32R = mybir.dt.float32r
BF16 = mybir.dt.bfloat16
I32 = mybir.dt.int32

# cpe (early-constant pack): tlo (f32) | w2s | ones (f32r)  (per-core)
# cpl (late-constant pack): [idm | iotaJ] (f32) then [tri | tris | onesc |
# iotaN | w2p] (f32r); _B marks offsets within the f32r SBUF tile.
_IDM = 0
_IOTAJ = 128
_F32R0 = 256            # start of the f32r half (DRAM cols)
_TRI = 256
_TRIS = 384
_ONESC = 416
_IOTAN = 418            # 2*NT cols, then w2p blocks


def _cpl_w(nt):
    return _IOTAN + 2 * nt + ((TPC + 3) // 4) * VT


def _w2p_off(nt, v):
    return _IOTAN + 2 * nt + (v // 4) * VT


def _round_fp32r(x):
    hi = x.astype(ml_dtypes.bfloat16).astype(np.float32)
    lo = (x - hi).astype(ml_dtypes.bfloat16).astype(np.float32)
    return hi + lo


def host_prep(logits, target, mask, word_vectors):
    """Compact masked rows, shard/stage the inputs into one map per core.

    Returns (in_maps, sel) where sel[i] = original row of compact row i.
    """
    mask = np.asarray(mask)
    sel = np.nonzero(mask)[0]
    n_eff = len(sel)
    # even tile count: fp32r matmuls need even moving free dims, so both
    # compaction ranges (h = nt-4 and nt-h) stay even
    nt = max((n_eff + 127) // 128, 2)
    nt += nt % 2
    npad = nt * 128

    x = np.zeros((npad, D), dtype=np.float32)
    x[:n_eff] = np.asarray(logits, dtype=np.float32)[sel]
    t = np.zeros((npad,), dtype=np.int64)
    t[:n_eff] = np.asarray(target).astype(np.int64)[sel]
    W = np.asarray(word_vectors, dtype=np.float32)

    # padded vocab: zero vectors with huge norm never count
    Wp = np.zeros((VPAD, D), dtype=np.float32)
    Wp[:V] = W
    w2 = np.zeros((VPAD,), dtype=np.float32)
    w2[:V] = (W.astype(np.float64) ** 2).sum(axis=1).astype(np.float32)
    w2[V:] = 1e30

    Wr = _round_fp32r(Wp)                                # [VPAD, D]
    w2r = _round_fp32r(w2)
    xm2r = _round_fp32r(np.ascontiguousarray(-2.0 * x))  # [NP, D] (-2x, fp32r)
    xb = (-2.0 * x).astype(ml_dtypes.bfloat16).astype(np.float32)  # bf16(-2x)
    Wb = Wp.astype(ml_dtypes.bfloat16).astype(np.float32)

    # host threshold, fp64: T[n] = w2r[t] - 2 x_r . w_r[t]  (sound w/ DELTA)
    T64 = (w2r[t].astype(np.float64)
           + np.einsum('nd,nd->n', xm2r.astype(np.float64),
                       Wr[t].astype(np.float64)))
    Tlo = (T64 - DELTA).astype(np.float32)

    def chunkT(a, cols):
        # [D, cols] -> [128, DC, cols]
        return np.ascontiguousarray(a.reshape(DC, 128, cols).transpose(1, 0, 2))

    xTb = chunkT(np.ascontiguousarray(xb.T), npad).astype(ml_dtypes.bfloat16)

    # early consts: tlo | w2s(per-core)
    tlo_t = np.ascontiguousarray(Tlo.reshape(nt, 128).T)  # [128, NT]

    # late consts (core-independent part)
    cpl = np.zeros((128, _cpl_w(nt)), dtype=np.float32)
    cpl[:, _IDM:_IDM + 128] = np.eye(128, dtype=np.float32)
    cpl[:, _IOTAJ:_IOTAJ + NSLOT] = np.tile(
        np.arange(NSLOT, dtype=np.float32), (128, 1))
    cpl[:, _TRI:_TRI + 128] = np.tril(
        np.ones((128, 128), dtype=np.float32), -1).T   # [p', p]=1 if p'<p
    cpl[:nt, _TRIS:_TRIS + nt] = np.tril(
        np.ones((nt, nt), dtype=np.float32), -1).T     # [i', i]=1 if i'<i
    cpl[:, _ONESC:_ONESC + 2] = 1.0
    iotaN1 = np.repeat(
        (np.arange(npad, dtype=np.float32) + 1.0).reshape(nt, 128).T[:, :, None],
        2, axis=2)  # [128, NT, 2] duplicated pair (fp32r needs even free)
    cpl[:, _IOTAN:_IOTAN + 2 * nt] = iotaN1.reshape(128, 2 * nt)

    # gather source: row n = [-2x (fp32r) | W[t] (fp32r) | w2[t] | pad]
    aug = np.zeros((npad, AUGW), dtype=np.float32)
    aug[:, :D] = xm2r
    aug[:, D:2 * D] = Wr[t]
    aug[:, 2 * D] = w2r[t]

    in_maps = []
    for c in range(NUM_CORES):
        sl = slice(c * VS, (c + 1) * VS)
        m = dict(xTb=xTb, aug=aug)
        m["wT"] = chunkT(np.ascontiguousarray(Wr[sl].T), VS)
        m["wsub"] = chunkT(np.ascontiguousarray(
            Wb[c * VS:c * VS + SUB_W].T), SUB_W).astype(ml_dtypes.bfloat16)
        cpe = np.zeros((128, nt + SUB_W + 128), dtype=np.float32)
        cpe[:, :nt] = tlo_t
        # screen w2 (exact fp32r values) at rows {0,32,64,96} (g = i%4)
        for g in range(4):
            cpe[32 * g, nt:nt + SUB_W] = w2r[c * VS:c * VS + SUB_W]
            cpe[32 * g, nt + SUB_W:nt + SUB_W + 128] = 1.0  # ones strips
        m["cpe"] = cpe
        cplc = cpl.copy()
        # refine w2 pack: row 32*(v%4), block v//4
        for v in range(TPC):
            cplc[32 * (v % 4), _w2p_off(nt, v):_w2p_off(nt, v) + VT] = \
                w2r[c * VS + v * VT:c * VS + (v + 1) * VT]
        m["cpl"] = cplc
        in_maps.append(m)
    return in_maps, sel


def make_io(nc, nt):
    npad = nt * 128
    ins = {
        "xTb": nc.dram_tensor("xTb", [128, DC, npad], BF16, kind="ExternalInput").ap(),
        "wT": nc.dram_tensor("wT", [128, DC, VS], F32, kind="ExternalInput").ap(),
        "wsub": nc.dram_tensor("wsub", [128, DC, SUB_W], BF16, kind="ExternalInput").ap(),
        "cpe": nc.dram_tensor("cpe", [128, nt + SUB_W + 128], F32, kind="ExternalInput").ap(),
        "cpl": nc.dram_tensor("cpl", [128, _cpl_w(nt)], F32, kind="ExternalInput").ap(),
        "aug": nc.dram_tensor("aug", [npad, AUGW], F32, kind="ExternalInput").ap(),
    }
    outs = {
        "riskyvals": nc.dram_tensor("riskyvals", [128, 1], F32, kind="ExternalOutput").ap(),
        "cntref": nc.dram_tensor("cntref", [128, TPC], F32, kind="ExternalOutput").ap(),
        "riskytot": nc.dram_tensor("riskytot", [1, 1], F32, kind="ExternalOutput").ap(),
    }
    return ins, outs


def build_nc(nt, num_cores=NUM_CORES):
    nc = bacc.Bacc("TRN2", target_bir_lowering=False, debug=False,
                   num_devices=num_cores)
    ins, outs = make_io(nc, nt)
    with tile.TileContext(nc, trace_sim=False) as tc:
        _knn_kernel(tc, ins, outs, nt)
    nc.compile()
    return nc


def _knn_kernel(tc, ins, outs, nt, repeats=1,
                phases=("screen", "compact", "gather", "refine")):
    nc = tc.nc
    ctx = ExitStack()
    with ctx:
        persist = ctx.enter_context(tc.tile_pool(name="persist", bufs=1))
        scratch = ctx.enter_context(tc.tile_pool(name="scratch", bufs=3))
        psm = ctx.enter_context(tc.tile_pool(name="psm", bufs=4, space="PSUM"))
        psg = ctx.enter_context(tc.tile_pool(name="psg", bufs=2, space="PSUM"))
        psc = ctx.enter_context(tc.tile_pool(name="psc", bufs=1, space="PSUM"))

        cpeA = persist.tile([128, nt], F32, tag="cpeA")
        cpeB = persist.tile([128, SUB_W + 128], F32R, tag="cpeB")
        cplA = persist.tile([128, _F32R0], F32, tag="cplA")
        cplB = persist.tile([128, _cpl_w(nt) - _F32R0], F32R, tag="cplB")
        for rep in range(repeats):
            # big inputs re-DMA'd per rep so the repeat-slope timing method
            # charges the HBM streaming to every iteration.
            if rep == 0:
                nc.sync.dma_start(cpeA[:], ins["cpe"][:, 0:nt])
                nc.sync.dma_start(
                    cpeB[:], ins["cpe"].bitcast(F32R)[:, nt:nt + SUB_W + 128])
            xTb_t = persist.tile([128, DC, nt * 128], BF16, tag="xTb")
            nq = (nt + 3) // 4 * 128
            sl = slice(0, nq)
            nc.sync.dma_start(xTb_t[:, :, sl], ins["xTb"][:, :, sl])
            wsub_b = persist.tile([128, DC, SUB_W], BF16, tag="wsub")
            nc.sync.dma_start(wsub_b[:, 0, :], ins["wsub"][:, 0, :])
            nc.sync.dma_start(wsub_b[:, 1:DC, :], ins["wsub"][:, 1:DC, :])
            for q in range(1, 4):
                sl = slice(q * nq, min((q + 1) * nq, nt * 128))
                if sl.start < sl.stop:
                    nc.sync.dma_start(xTb_t[:, :, sl], ins["xTb"][:, :, sl])
            if rep == 0:
                nc.sync.dma_start(cplA[:], ins["cpl"][:, 0:_F32R0])
                nc.sync.dma_start(
                    cplB[:], ins["cpl"].bitcast(F32R)[:, _F32R0:_cpl_w(nt)])
            wT_r = persist.tile([128, DC, VS], F32R, tag="wT")
            half = (7 * VT + WT_TAIL * VT) // 2
            for c0, c1 in ((0, 2 * VT), (2 * VT, 4 * VT), (4 * VT, 6 * VT),
                           (6 * VT, 7 * VT), (7 * VT, half),
                           (half, WT_TAIL * VT)):
                sl = slice(c0, c1)
                nc.sync.dma_start(wT_r[:, :, sl], ins["wT"].bitcast(F32R)[:, :, sl])

            def emit_wt_tail(_done=[False]):
                if _done[0]:
                    return
                _done[0] = True
                for v0, v1 in ((WT_TAIL, 10), (10, 12), (12, TPC)):
                    sl = slice(v0 * VT, v1 * VT)
                    nc.sync.dma_start(wT_r[:, :, sl],
                                      ins["wT"].bitcast(F32R)[:, :, sl])

            _knn_body(tc, ins, outs, nt, persist, scratch, psm, psg, psc,
                      cpeA, cpeB, cplA, cplB, wsub_b, xTb_t, wT_r,
                      emit_wt_tail, phases=phases)
            emit_wt_tail()
        if "refine" not in phases:
            # ablation builds: touch the outputs so the NEFF has writers
            dummy = persist.tile([128, TPC], F32, tag="dummy")
            nc.gpsimd.memset(dummy[:], 0.0)
            nc.sync.dma_start(outs["cntref"][:], dummy[:])
            if "compact" not in phases:
                nc.sync.dma_start(outs["riskyvals"][:], dummy[:, 0:1])
                dummy1 = persist.tile([1, 1], F32, tag="dummy1")
                nc.gpsimd.memset(dummy1[:], 0.0)
                nc.sync.dma_start(outs["riskytot"][:], dummy1[:])
            # consume the big tiles so their DMAs aren't dead-code
            sink = psc.tile([128, 64], F32, tag="pc", name="sink")
            nc.tensor.matmul(sink[0:128, 0:32], wT_r[:, 0, 0:128],
                             wT_r[:, 0, VS - 32:VS], start=True, stop=True)
            nc.tensor.matmul(sink[0:128, 0:32], xTb_t[:, 0, 0:128],
                             xTb_t[:, 0, 0:32], start=False, stop=False)
            nc.tensor.matmul(sink[0:128, 0:32], wsub_b[:, 0, 0:128],
                             wsub_b[:, 0, 0:32], start=False, stop=True)
            sinks = persist.tile([128, 32], F32, tag="sinks")
            nc.vector.tensor_copy(sinks[:], sink[0:128, 0:32])


def _knn_body(tc, ins, outs, nt, persist, scratch, psm, psg, psc,
              cpeA, cpeB, cplA, cplB, wsub_b, xTb_t, wT_r, emit_wt_tail,
              phases=("screen", "compact", "gather", "refine")):
    nc = tc.nc
    if "screen" not in phases:
        return

    # constant views
    tlo_t = cpeA[:, 0:nt]
    w2s_r = cpeB[:, 0:SUB_W]
    ones_r = cpeB[:, SUB_W:SUB_W + 128]
    idm_t = cplA[:, _IDM:_IDM + 128]
    iotaJ_t = cplA[:, _IOTAJ:_IOTAJ + NSLOT]
    tri_r = cplB[:, _TRI - _F32R0:_TRI - _F32R0 + 128]
    tris_r = cplB[:, _TRIS - _F32R0:_TRIS - _F32R0 + nt]
    onesc_r = cplB[:, _ONESC - _F32R0:_ONESC - _F32R0 + 2]

    def iotaN1_r(i):
        o = _IOTAN - _F32R0 + 2 * i
        return cplB[:, o:o + 2]

    def w2p_r(v):
        g = 32 * (v % 4)
        o = _w2p_off(nt, v) - _F32R0
        return cplB[g:g + 32, o:o + VT]

    do_compact = "compact" in phases
    cnts = persist.tile([128, nt], F32, tag="cnts")
    Fr = persist.tile([128, nt], F32R, tag="Fr")
    Fo = persist.tile([128, nt], F32, tag="Fo")
    cp_r = persist.tile([1, nt], F32R, tag="cp_r")
    off = persist.tile([128, nt], F32, tag="off")
    tot1 = persist.tile([1, 1], F32, tag="tot1")
    tot_sb = persist.tile([1, 1], F32, tag="tot_sb")
    idx_ps = psg.tile([128, 256], F32, tag="px", name="idx_ps")

    filsink = persist.tile([128, 2], F32, tag="filsink")

    def pe_filler(n_mm, name, dep=None):
        # keep-warm matmuls; the DVE copy keeps them out of DCE.  `dep`
        # pins them behind a data dependency so the readiness-based
        # scheduler cannot hoist them out of the idle window they bridge.
        fil = psc.tile([128, 64], F32, tag="fil", name=name)
        stat = dep if dep is not None else ones_r
        m = stat.shape[-1]
        for _ in range(n_mm):
            nc.tensor.matmul(fil[0:m, 0:64], stat, ones_r[:, 0:64],
                             start=True, stop=True)
        nc.vector.tensor_copy(filsink[0:m, :], fil[0:m, 0:2])

    def screen_tiles(i0, i1, eng):
        # ---- phase 1: screen — one PSUM bank per i-tile; exact is_lt
        # count on DVE for [0, h), sign-trick estimate on ACT for the last
        # tiles (frees the DVE for the overlapped compaction chain).
        for i in range(i0, i1):
            pm = psm.tile([128, SUB_W], F32, tag="pm", name="pm")
            for d in range(DC):
                nc.tensor.matmul(pm[:], xTb_t[:, d, i * 128:(i + 1) * 128],
                                 wsub_b[:, d, :], start=(d == 0), stop=False)
            g = 32 * (i % 4)
            nc.tensor.matmul(pm[:], ones_r[g:g + 32, :], w2s_r[g:g + 32, :],
                             start=False, stop=True, tile_position=(g, 0))
            if eng == "dve":
                cmp = scratch.tile([128, SUB_W], F32, tag="cmp", name="cmp")
                nc.vector.tensor_scalar(
                    cmp[:], pm[:], tlo_t[:, i:i + 1], None,
                    op0=mybir.AluOpType.is_lt, op1=mybir.AluOpType.add,
                    accum_out=cnts[:, i:i + 1])
            else:
                sg = scratch.tile([128, SUB_W], BF16, tag="sg", name="sg")
                nc.scalar.activation(
                    sg[:], pm[:], mybir.ActivationFunctionType.Sign,
                    bias=tlo_t[:, i:i + 1], scale=-1.0,
                    accum_out=cnts[:, i:i + 1])

    def compact_range(i0, i1, thr):
        # ---- phase 2: risky flags, prefix sum, slot list for [i0, i1) ---
        # thr: RISK_THR on exact DVE counts, ACT_THR on raw ACT sign-sums
        # (acc = 2*c_lt - W + c_eq, so c_lt + c_eq/2 < R  <=>  acc < 2R-W).
        w = i1 - i0
        first, last = i0 == 0, i1 == nt
        nc.vector.tensor_scalar(Fr[:, i0:i1], cnts[:, i0:i1], thr, None,
                                op0=mybir.AluOpType.is_lt)
        # Fo = 0 for risky rows, 2*NSLOT (out of range) otherwise
        nc.vector.tensor_scalar(Fo[:, i0:i1], cnts[:, i0:i1], thr,
                                float(2 * NSLOT),
                                op0=mybir.AluOpType.is_ge,
                                op1=mybir.AluOpType.mult)
        # column sums (transposed): cs[i] = sum_p F[p, i]
        pc1 = psc.tile([128, 64], F32, tag="pc", name="pc1")
        nc.tensor.matmul(pc1[0:w, 0:2], Fr[:, i0:i1], onesc_r[:],
                         start=True, stop=True)
        cs_r = persist.tile([w, 2], F32R, tag="cs_%d" % i0)
        nc.vector.tensor_copy(cs_r[:], pc1[0:w, 0:2])
        # exclusive prefix over columns (+ carry from the previous range)
        pc3 = psc.tile([128, 64], F32, tag="pc", name="pc3")
        nc.tensor.matmul(pc3[0:2, 0:w], cs_r[:], tris_r[0:w, 0:w],
                         start=True, stop=True)
        if first:
            nc.vector.tensor_copy(cp_r[0:1, i0:i1], pc3[0:1, 0:w])
        else:
            nc.vector.tensor_scalar(cp_r[0:1, i0:i1], pc3[0:1, 0:w],
                                    tot1[0:1, 0:1], None,
                                    op0=mybir.AluOpType.add)
        # range total (carry / overflow detection)
        pc2 = psc.tile([128, 64], F32, tag="pc", name="pc2")
        nc.tensor.matmul(pc2[0:2, 0:2], cs_r[:], onesc_r[0:w, :],
                         start=True, stop=True)
        tdst = tot1 if not last else tot_sb
        if first:
            nc.vector.tensor_copy(tdst[:], pc2[0:1, 0:1])
        else:
            nc.vector.tensor_scalar(tdst[:], pc2[0:1, 0:1], tot1[0:1, 0:1],
                                    None, op0=mybir.AluOpType.add)
        if last:
            nc.sync.dma_start(outs["riskytot"][:], tot_sb[:])
        # global exclusive prefix P[p, i] = cp[i] + sum_{p'<p} F[p', i]
        P_ps = psc.tile([128, 64], F32, tag="pc", name="P_ps")
        nc.tensor.matmul(P_ps[:, 0:w], tri_r[:], Fr[:, i0:i1],
                         start=True, stop=False)
        nc.tensor.matmul(P_ps[:, 0:w], ones_r[0:1, :], cp_r[0:1, i0:i1],
                         start=False, stop=True)
        # off = P + Fo: slot for risky rows, out-of-range otherwise
        nc.vector.tensor_tensor(off[:, i0:i1], P_ps[:, 0:w], Fo[:, i0:i1],
                                op=mybir.AluOpType.add)
        # slot list: sel_i[p, j] = [off[p, i] == j]; with sel_i STATIONARY,
        # idx_ps[j, 0:2] += sum_p sel_i[p, j]*(n+1) puts the slot index on
        # the partition axis directly (no transposes needed).
        for i in range(i0, i1):
            sel_i = scratch.tile([128, NSLOT], F32R, tag="seli", name="seli")
            nc.vector.tensor_tensor(
                sel_i[:], iotaJ_t[:],
                off[:, i:i + 1].to_broadcast([128, NSLOT]),
                op=mybir.AluOpType.is_equal)
            nc.tensor.matmul(idx_ps[0:NSLOT, 0:2], sel_i[:], iotaN1_r(i),
                             start=(i == 0), stop=(i == nt - 1))

    h = nt - 4 if (do_compact and nt >= 8) else nt
    pe_filler(WARM_MM, "warm")   # ramp the PE during the input-DMA head
    screen_tiles(0, h, "dve")
    if do_compact and h < nt:
        compact_range(0, h, RISK_THR)  # overlaps the screen's last i-tiles
        screen_tiles(h, nt, "act")
        compact_range(h, nt, ACT_THR)
    elif do_compact:
        compact_range(0, nt, RISK_THR)
    if not do_compact:
        return

    idxv = persist.tile([128, 1], F32, tag="idxv")
    nc.vector.tensor_copy(idxv[:], idx_ps[0:NSLOT, 0:1])
    nc.sync.dma_start(outs["riskyvals"][:], idxv[:])
    idxg = persist.tile([128, 1], I32, tag="idxg")
    nc.vector.tensor_scalar(idxg[:], idx_ps[0:NSLOT, 1:2], 1.0, 1.0,
                            op0=mybir.AluOpType.max,
                            op1=mybir.AluOpType.subtract)

    if "gather" not in phases:
        return
    # ---- phase 3: gather risky rows + transpose to stationary layout ----
    # aug row: x chunks 0-3, wg chunks 4-7, w2 at col 2D (chunk 8, 8 wide)
    xgT = persist.tile([128, DC, 128], F32R, tag="xgT")
    wgT = persist.tile([128, DC + 1, 128], F32R, tag="wgT")
    g_sb = scratch.tile([128, AUGW], F32, tag="gsb", name="gsb")
    nc.gpsimd.indirect_dma_start(
        out=g_sb[:], out_offset=None, in_=ins["aug"][:],
        in_offset=bass.IndirectOffsetOnAxis(ap=idxg[:, 0:1], axis=0))
    emit_wt_tail()
    # PE filler: keeps the PE p-state warm across the gather DMA wait
    pe_filler(FILL_MM, "fill", dep=Fr[:, nt - 2:nt])
    evac = [(ch, xgT[:, ch, :]) for ch in range(DC)]
    evac += [(DC + ch, wgT[:, ch, :]) for ch in range(DC)]
    for k, (ch, dst_ap) in enumerate(evac):
        tp = psg.tile([128, 256], F32, tag="px", name="tp")
        nc.tensor.transpose(tp[:, 0:128], g_sb[:, ch * 128:(ch + 1) * 128],
                            idm_t[:])
        if k % 2 == 0:
            nc.vector.tensor_copy(dst_ap, tp[:, 0:128])
        else:
            nc.scalar.activation(dst_ap, tp[:, 0:128],
                                 mybir.ActivationFunctionType.Copy)
    # w2 row: transpose the 8-wide pad block, keep partition row 0 (the
    # Tg matmul's stationary is a single ones-row, so junk rows are unread)
    tpw = psg.tile([128, 256], F32, tag="px", name="tpw")
    nc.tensor.transpose(tpw[0:8, 0:128], g_sb[:, 2 * D:2 * D + 8], idm_t[:])
    nc.vector.tensor_copy(wgT[0:1, DC, :], tpw[0:1, 0:128])

    # ---- phase 4: exact threshold extraction for gathered rows ----------
    Tg = persist.tile([128, 1], F32, tag="Tg")
    pg = psg.tile([128, 256], F32, tag="px", name="pg")
    for d in range(DC):
        nc.tensor.matmul(pg[:, 0:128], xgT[:, d, :], wgT[:, d, :],
                         start=(d == 0), stop=False)
    nc.tensor.matmul(pg[:, 0:128], ones_r[0:1, :], wgT[0:1, DC, :],
                     start=False, stop=True, tile_position=(0, 0))
    scr = scratch.tile([128, 128], F32, tag="scr", name="scr")
    nc.vector.tensor_tensor(scr[:], pg[:, 0:128], idm_t[:],
                            op=mybir.AluOpType.mult)
    nc.vector.tensor_reduce(Tg[:, 0:1], scr[:],
                            axis=mybir.AxisListType.X,
                            op=mybir.AluOpType.add)

    if "refine" not in phases:
        return
    # ---- phase 5: refine — exact fp32r counts over the full shard -------
    # counts: even v-tiles exact on DVE (is_lt), odd v-tiles on ACT via the
    # sign trick; the host combines (act_sum + N_ODD*VT)/2 with a -0.5
    # self-pair correction for the core that owns the row's target word.
    cref = persist.tile([128, TPC], F32, tag="cref")
    for v0 in range(0, TPC, 2):
        vs = list(range(v0, min(v0 + 2, TPC)))
        pmr = {}
        for v in vs:
            pm = psm.tile([128, SUB_W], F32, tag="pm", name="pmr")
            pmr[v] = pm
            for d in range(DC):
                nc.tensor.matmul(pm[:, :VT], xgT[:, d, :],
                                 wT_r[:, d, v * VT:(v + 1) * VT],
                                 start=(d == 0), stop=False)
        for v in vs:
            g = 32 * (v % 4)
            nc.tensor.matmul(pmr[v][:, :VT], ones_r[g:g + 32, :], w2p_r(v),
                             start=False, stop=True, tile_position=(g, 0))
        for v in vs:
            if v % 2 == 0:
                cmp = scratch.tile([128, VT], F32, tag="cmpr", name="cmpr")
                nc.vector.tensor_scalar(
                    cmp[:], pmr[v][:, :VT], Tg[:, 0:1], None,
                    op0=mybir.AluOpType.is_lt, op1=mybir.AluOpType.add,
                    accum_out=cref[:, v:v + 1])
            else:
                sgr = scratch.tile([128, VT], BF16, tag="sgr", name="sgr")
                nc.scalar.activation(
                    sgr[:], pmr[v][:, :VT],
                    mybir.ActivationFunctionType.Sign,
                    bias=Tg[:, 0:1], scale=-1.0,
                    accum_out=cref[:, v:v + 1])
    nc.sync.dma_start(outs["cntref"][:], cref[:])


_NC_CACHE = {}


def _get_nc(nt):
    if nt not in _NC_CACHE:
        _NC_CACHE[nt] = build_nc(nt)
    return _NC_CACHE[nt]


def kernel(logits, target, mask, word_vectors):
    """Full inputs in, full output out (shape [1] float32)."""
    from concourse.bass_utils import run_bass_kernel_spmd

    in_maps, sel = host_prep(logits, target, mask, word_vectors)
    nt = in_maps[0]["xTb"].shape[2] // 128
    n_eff = len(sel)
    nc = _get_nc(nt)

    last_err = None
    res = None
    for attempt in range(3):
        try:
            res = run_bass_kernel_spmd(nc, in_maps, list(range(NUM_CORES)))
            break
        except Exception as e:  # transient NRT/axon failures: retry
            last_err = e
    if res is None:
        raise last_err

    # host combine: row is a hit iff refined on every core and sum(cnt) < K
    mask = np.asarray(mask).astype(np.float64)
    tgt = np.asarray(target).astype(np.int64)
    totals = {}
    present = {}
    for c in range(NUM_CORES):
        r = res.results[c]
        assert float(np.asarray(r["riskytot"]).reshape(-1)[0]) <= NSLOT, \
            "risky row overflow — NSLOT too small"
        vals = np.asarray(r["riskyvals"]).reshape(128)
        cnt = np.asarray(r["cntref"]).reshape(128, TPC)
        for j in range(NSLOT):
            v = int(round(float(vals[j])))
            if v <= 0 or v > n_eff:
                continue
            n = v - 1   # compact row index
            raw = (float(cnt[j, 0:TPC:2].sum())
                   + (float(cnt[j, 1:TPC:2].sum()) + N_ODD * VT) / 2.0)
            tn = int(tgt[sel[n]])
            if c * VS <= tn < (c + 1) * VS and ((tn - c * VS) // VT) % 2 == 1:
                raw -= 0.5  # self-pair (s == T exactly) in an ACT tile
            assert abs(raw - round(raw)) < 1e-3, \
                f"non-integer refined count {raw} (unexpected exact tie)"
            totals[n] = totals.get(n, 0.0) + round(raw)
            present[n] = present.get(n, 0) + 1
    hits = 0.0
    for n, p in present.items():
        if p == NUM_CORES and totals[n] < K:
            hits += 1.0
    acc = hits / mask.sum()
    return np.asarray([acc], dtype=np.float32)
